# revision 23
# baseline (speedup 1.0000x reference)
"""Bilinear resampling (tf-resampler semantics) on 8 TRN2 NeuronCores.

out[b,y,x] = bilinear_sample(imgs[b], y + dvfs[b,y,x,1], x + dvfs[b,y,x,0])
with zero-padding for out-of-bounds corners.

Strategy: pure data-parallel over batch (4 images per core).  On-chip the
per-pixel 2D gather is computed as a dense separable "hat" select-sum:

    out[y,x] = sum_r hat(dy-r) * sum_c hat(dx-c) * I[y+r, x+c]

where hat(t) = max(0, 1-|t|).  hat(dy-r)*hat(dx-c) is exactly the bilinear
corner weight for corner (y+r, x+c) and is nonzero only for the 4 corners
of each pixel, so summing over a tap set that covers all occurring
(floor(dy), floor(dx)) pairs is exact.  The tap set is computed on the host
from the actual displacement field (cheap histogram) so device work scales
with the true support of the data.

Implementation notes:
- Images are zero-padded on the host; each 128-row tile loads all n_r
  row-shifted copies of the image window with ONE overlapping 3D-AP DMA
  (engine APs require partition base in {0,32,64,96}, so row shifts cannot
  be partition-offset views; DMA has no such restriction).
- Hat coefficient planes are built on the Scalar (ACT) engine; tap
  multiply/adds run on the Vector (DVE) engine.
"""

import sys

sys.path.insert(0, "/opt/trn_rl_repo")

import dataclasses
from contextlib import ExitStack

import numpy as np

import concourse.bass as bass
import concourse.mybir as mybir
from concourse import tile
from concourse.bass_utils import run_bass_kernel_spmd

F32 = mybir.dt.float32
BF16 = mybir.dt.bfloat16
AF = mybir.ActivationFunctionType

N_CORES = 8

# compute taps in bf16 (2x DVE throughput); coefficients and accumulation
# structured so the result stays well inside a 2e-2 scale-relative gate
COMPUTE_BF16 = True


def _tap_sets(dvfs):
    """Exact tap support from the data: {r: sorted list of c}, for taps
    (r, c) such that some pixel has floor(dy) in {r-1, r} and
    floor(dx) in {c-1, c}."""
    fx = np.floor(dvfs[..., 0]).astype(np.int64).ravel()
    fy = np.floor(dvfs[..., 1]).astype(np.int64).ravel()
    lo = int(min(fx.min(), fy.min()))
    hi = int(max(fx.max(), fy.max()))
    n = hi - lo + 1
    joint = np.bincount((fy - lo) * n + (fx - lo), minlength=n * n) > 0
    joint = joint.reshape(n, n)
    # tap (r, c) needed iff joint[r - dr, c - dc] for (dr, dc) in {0,1}^2
    need = np.zeros((n + 1, n + 1), dtype=bool)
    for dr in (0, 1):
        for dc in (0, 1):
            need[dr : dr + n, dc : dc + n] |= joint
    taps = {}
    for ri in range(n + 1):
        cs = [int(ci) + lo for ci in np.nonzero(need[ri])[0]]
        if cs:
            taps[ri + lo] = cs
    return taps


def _split_multi_waits(nc):
    """This stack's walrus accepts at most one sync-wait per instruction;
    Tile emits several.  Hoist all-but-one wait onto preceding NoOps on the
    same engine queue (sequential execution makes that equivalent)."""
    for fn in nc.m.functions:
        for blk in fn.blocks:
            new_insts = []
            for ins in blk.instructions:
                si = ins.sync_info
                if si is not None and si.on_wait and len(si.on_wait) > 1:
                    waits = list(si.on_wait)
                    for w in waits[:-1]:
                        new_insts.append(
                            mybir.InstNoOp(
                                name=nc.get_next_instruction_name(),
                                engine=ins.engine,
                                bass_nofuse=True,
                                sync_info=mybir.SyncInfo(
                                    on_wait=[w], on_update=[]
                                ),
                            )
                        )
                    si.on_wait = [waits[-1]]
                new_insts.append(ins)
            blk.instructions = new_insts


def _build(taps, n_imgs, H, W):
    rs_used = sorted(taps.keys())
    rmin, rmax = min(rs_used), max(rs_used)
    # per-r contiguous c-ranges with even start: with pad_l and Wp even,
    # every bf16 tap view is then 4-byte aligned so the DVE 2x perf mode can
    # engage.  Extra taps added by widening are exactly zero-weighted.
    c_range = {}
    for r in rs_used:
        c0, c1 = min(taps[r]), max(taps[r])
        if c0 % 2 != 0:
            c0 -= 1
        c_range[r] = (c0, c1)
    cmin = min(c0 for c0, _ in c_range.values())
    cmax = max(c1 for _, c1 in c_range.values())
    pad_t, pad_b = max(0, -rmin), max(0, rmax)
    pad_l, pad_r = max(0, -cmin), max(0, cmax)
    n_r = rmax - rmin + 1
    Hp, Wp = H + pad_t + pad_b, W + pad_l + pad_r
    if Wp % 2 == 1:
        pad_r += 1
        Wp += 1
    cs_union = list(range(cmin, cmax + 1))
    c_idx = {c: i for i, c in enumerate(cs_union)}
    j_of_r = {r: r - rmin for r in rs_used}

    nc = bass.Bass()
    imgs = nc.dram_tensor("imgs", [n_imgs, Hp, Wp], F32, kind="ExternalInput")
    dvfs = nc.dram_tensor("dvfs", [n_imgs, H, 2 * W], F32, kind="ExternalInput")
    out = nc.dram_tensor("out", [n_imgs, H, W], F32, kind="ExternalOutput")

    # activation() biases must come from the const-AP registry
    consts = sorted({-float(v) for v in set(rs_used) | set(cs_union)} - {0.0, 1.0})
    for v in consts:
        t = nc.alloc_sbuf_tensor(f"const-f32-{v}", [128, 1], F32)
        nc.gpsimd.memset(t.ap(), v)
        nc.const_aps.aps[(F32, v)] = t.ap()
    nc.all_engine_barrier()

    with ExitStack() as ctx:
        tc = ctx.enter_context(tile.TileContext(nc))
        img_pool = ctx.enter_context(tc.tile_pool(name="img", bufs=1))
        dvf_pool = ctx.enter_context(tc.tile_pool(name="dvf", bufs=2))
        bp_pool = ctx.enter_context(tc.tile_pool(name="bp", bufs=1))
        av_pool = ctx.enter_context(tc.tile_pool(name="av", bufs=2))
        u_pool = ctx.enter_context(tc.tile_pool(name="u", bufs=2))
        h_pool = ctx.enter_context(tc.tile_pool(name="h", bufs=2))
        t_pool = ctx.enter_context(tc.tile_pool(name="t", bufs=1))
        o_pool = ctx.enter_context(tc.tile_pool(name="o", bufs=2))

        CD = BF16 if COMPUTE_BF16 else F32
        ib_pool = ctx.enter_context(tc.tile_pool(name="ib", bufs=1))

        for b in range(n_imgs):
            for t0 in range(0, H, 128):
                # all n_r row-shifted image windows in one overlapping DMA:
                # IRALL[p, j*Wp + u] = imgs_padded[b, t0 + j + p, u]
                if not COMPUTE_BF16:
                    IRALL = img_pool.tile([128, n_r * Wp], F32, tag="IR")
                    src = dataclasses.replace(
                        imgs[b],
                        ap=[[Wp, 128], [Wp, n_r], [1, Wp]],
                        offset=b * Hp * Wp + t0 * Wp,
                    )
                    dst = IRALL[:, :].rearrange("p (j w) -> p j w", j=n_r)
                    nc.sync.dma_start(out=dst, in_=src)
                    IB = IRALL
                else:
                    # stage fp32 halves through a small buffer, convert to a
                    # bf16 copy (taps then run in the DVE 2x perf mode)
                    IB = ib_pool.tile([128, n_r * Wp], BF16, tag="IB")
                    nh = (n_r + 1) // 2
                    for hj, (j0, j1) in enumerate(((0, nh), (nh, n_r))):
                        if j1 <= j0:
                            continue
                        stage = img_pool.tile([128, nh * Wp], F32, tag="IR")
                        src = dataclasses.replace(
                            imgs[b],
                            ap=[[Wp, 128], [Wp, j1 - j0], [1, Wp]],
                            offset=b * Hp * Wp + (t0 + j0) * Wp,
                        )
                        dst = stage[:, 0 : (j1 - j0) * Wp].rearrange(
                            "p (j w) -> p j w", j=j1 - j0
                        )
                        nc.sync.dma_start(out=dst, in_=src)
                        nc.scalar.activation(
                            IB[:, j0 * Wp : j1 * Wp],
                            stage[:, 0 : (j1 - j0) * Wp],
                            AF.Copy,
                            bias=0.0,
                            scale=1.0,
                        )

                D = dvf_pool.tile([128, 2 * W], F32, tag="D")
                nc.sync.dma_start(out=D[:, :], in_=dvfs[b, t0 : t0 + 128, :])
                dx = D[:, 0 : 2 * W : 2]
                dy = D[:, 1 : 2 * W : 2]

                # horizontal hat planes: B_c = relu(1 - |dx - c|)
                BP = bp_pool.tile([128, W * len(cs_union)], CD, tag="BP")
                for c in cs_union:
                    i = c_idx[c]
                    U = u_pool.tile([128, W], F32, tag="U")
                    nc.scalar.activation(U[:, :], dx, AF.Abs, bias=-float(c), scale=1.0)
                    nc.scalar.activation(
                        BP[:, i * W : (i + 1) * W],
                        U[:, :],
                        AF.Relu,
                        bias=1.0,
                        scale=-1.0,
                    )

                OUT = o_pool.tile([128, W], F32, tag="OUT")
                max_nc = max(c1 - c0 + 1 for c0, c1 in c_range.values())
                first_r = True
                for r in rs_used:
                    # vertical hat plane: A_r = relu(1 - |dy - r|)
                    U = u_pool.tile([128, W], F32, tag="U")
                    AV = av_pool.tile([128, W], CD, tag="AV")
                    nc.scalar.activation(U[:, :], dy, AF.Abs, bias=-float(r), scale=1.0)
                    nc.scalar.activation(
                        AV[:, :], U[:, :], AF.Relu, bias=1.0, scale=-1.0
                    )

                    j = j_of_r[r]
                    c0, c1 = c_range[r]
                    k = c1 - c0 + 1
                    # all k taps of this row in ONE mult: in0 is an
                    # overlapping [128, k, W] view of the shifted image row,
                    # in1 the matching stack of hat planes
                    T = t_pool.tile([128, max_nc * W], CD, tag="T")
                    iv = dataclasses.replace(
                        IB[:, :],
                        ap=[[n_r * Wp, 128], [1, k], [1, W]],
                        offset=j * Wp + pad_l + c0,
                    )
                    bp = dataclasses.replace(
                        BP[:, :],
                        ap=[[len(cs_union) * W, 128], [W, k], [1, W]],
                        offset=c_idx[c0] * W,
                    )
                    tv = T[:, 0 : k * W].rearrange("p (k w) -> p k w", k=k)
                    nc.vector.tensor_tensor(tv, iv, bp, mybir.AluOpType.mult)
                    # fold the k product planes down to one
                    while k > 1:
                        if k % 2 == 1:
                            nc.vector.tensor_add(
                                T[:, 0:W], T[:, 0:W], T[:, (k - 1) * W : k * W]
                            )
                            k -= 1
                        else:
                            h = k // 2
                            nc.vector.tensor_add(
                                T[:, 0 : h * W],
                                T[:, 0 : h * W],
                                T[:, h * W : k * W],
                            )
                            k = h
                    # vertical blend on GpSimd (otherwise idle)
                    if first_r:
                        nc.gpsimd.tensor_mul(OUT[:, :], T[:, 0:W], AV[:, :])
                        first_r = False
                    else:
                        T2 = h_pool.tile([128, W], CD, tag="H")
                        nc.gpsimd.tensor_mul(T2[:, :], T[:, 0:W], AV[:, :])
                        nc.gpsimd.tensor_add(OUT[:, :], OUT[:, :], T2[:, :])

                nc.sync.dma_start(out=out[b, t0 : t0 + 128, :], in_=OUT[:, :])

    _split_multi_waits(nc)
    pads = (pad_t, pad_b, pad_l, pad_r)
    return nc, pads


def _make_runner(nc):
    """Mirror of bass2jax.run_bass_via_pjrt's multi-core path, but returning
    a reusable jitted callable so the NEFF can be re-executed for timing."""
    import jax
    from jax.experimental.shard_map import shard_map
    from jax.sharding import Mesh, PartitionSpec

    from concourse import bass2jax, mybir as mb

    bass2jax.install_neuronx_cc_hook()
    partition_name = nc.partition_id_tensor.name if nc.partition_id_tensor else None
    in_names, out_names, out_avals, zero_outs = [], [], [], []
    for alloc in nc.m.functions[0].allocations:
        if not isinstance(alloc, mb.MemoryLocationSet):
            continue
        name = alloc.memorylocations[0].name
        if alloc.kind == "ExternalInput":
            if name != partition_name:
                in_names.append(name)
        elif alloc.kind == "ExternalOutput":
            out_names.append(name)
            shape = tuple(alloc.tensor_shape)
            dtype = mb.dt.np(alloc.dtype)
            out_avals.append(jax.core.ShapedArray(shape, dtype))
            zero_outs.append(np.zeros(shape, dtype))
    n_params = len(in_names)
    n_outs = len(out_avals)
    all_in_names = list(in_names) + list(out_names)
    if partition_name is not None:
        all_in_names.append(partition_name)
    donate = tuple(range(n_params, n_params + n_outs))

    def _body(*args):
        operands = list(args)
        if partition_name is not None:
            operands.append(bass2jax.partition_id_tensor())
        outs = bass2jax._bass_exec_p.bind(
            *operands,
            out_avals=tuple(out_avals),
            in_names=tuple(all_in_names),
            out_names=tuple(out_names),
            lowering_input_output_aliases=(),
            sim_require_finite=True,
            sim_require_nnan=True,
            nc=nc,
        )
        return tuple(outs)

    devices = jax.devices()[:N_CORES]
    mesh = Mesh(np.asarray(devices), ("core",))
    in_specs = (PartitionSpec("core"),) * (n_params + n_outs)
    out_specs = (PartitionSpec("core"),) * n_outs
    # no donation: the kernel writes every output element, so the "zero"
    # output buffers can be staged on device once and reused across calls
    sharded = jax.jit(
        shard_map(
            _body, mesh=mesh, in_specs=in_specs, out_specs=out_specs, check_rep=False
        ),
        keep_unused=True,
    )

    from jax.sharding import NamedSharding

    shd = NamedSharding(mesh, PartitionSpec("core"))

    def run(in_maps, materialize=True, _staged={}):
        key = id(in_maps)
        if key not in _staged:
            per_core = [[np.asarray(m[name]) for name in in_names] for m in in_maps]
            concat_in = [
                np.concatenate([per_core[c][i] for c in range(N_CORES)], axis=0)
                for i in range(n_params)
            ]
            concat_zeros = [
                np.zeros((N_CORES * z.shape[0], *z.shape[1:]), z.dtype)
                for z in zero_outs
            ]
            _staged.clear()
            _staged[key] = [
                jax.device_put(a, shd) for a in concat_in + concat_zeros
            ]
            jax.block_until_ready(_staged[key])
        args = _staged[key]
        out_arrs = sharded(*args)
        jax.block_until_ready(out_arrs)
        if not materialize:
            return None
        return [
            {
                name: np.asarray(out_arrs[i]).reshape(N_CORES, *out_avals[i].shape)[c]
                for i, name in enumerate(out_names)
            }
            for c in range(N_CORES)
        ]

    return run


def _null_runner():
    """Tiny copy kernel used to measure fixed per-call dispatch overhead."""
    nc = bass.Bass()
    x = nc.dram_tensor("x", [128, 128], F32, kind="ExternalInput")
    y = nc.dram_tensor("y", [128, 128], F32, kind="ExternalOutput")
    from contextlib import ExitStack

    with ExitStack() as ctx:
        tc = ctx.enter_context(tile.TileContext(nc))
        pool = ctx.enter_context(tc.tile_pool(name="p", bufs=1))
        t = pool.tile([128, 128], F32)
        nc.sync.dma_start(out=t[:, :], in_=x[:, :])
        nc.sync.dma_start(out=y[:, :], in_=t[:, :])
    _split_multi_waits(nc)
    runner = _make_runner(nc)
    in_maps = [{"x": np.zeros((128, 128), np.float32)} for _ in range(N_CORES)]
    return runner, in_maps


def _prepare(imgs, dvfs):
    imgs = np.ascontiguousarray(np.asarray(imgs, dtype=np.float32))
    dvfs = np.ascontiguousarray(np.asarray(dvfs, dtype=np.float32))
    B, H, W = imgs.shape[0], imgs.shape[1], imgs.shape[2]
    n_per = B // N_CORES
    taps = _tap_sets(dvfs)
    nc, pads = _build(taps, n_per, H, W)
    pad_t, pad_b, pad_l, pad_r = pads
    imgs_p = np.zeros((B, H + pad_t + pad_b, W + pad_l + pad_r), np.float32)
    imgs_p[:, pad_t : pad_t + H, pad_l : pad_l + W] = imgs.reshape(B, H, W)
    dvfs3 = dvfs.reshape(B, H, 2 * W)
    in_maps = [
        {
            "imgs": imgs_p[i * n_per : (i + 1) * n_per],
            "dvfs": dvfs3[i * n_per : (i + 1) * n_per],
        }
        for i in range(N_CORES)
    ]
    return nc, in_maps, (B, H, W)


def _run(imgs, dvfs):
    nc, in_maps, (B, H, W) = _prepare(imgs, dvfs)
    runner = _make_runner(nc)
    results = runner(in_maps)
    outs = [np.asarray(m["out"]) for m in results]
    full = np.concatenate(outs, axis=0).reshape(B, H, W, 1)
    return full, runner, in_maps


def kernel(**inputs):
    full, _, _ = _run(inputs["imgs"], inputs["dvfs"])
    return full


# revision 24
# speedup vs baseline: 1.3379x; 1.3379x over previous
"""Bilinear resampling (tf-resampler semantics) on 8 TRN2 NeuronCores.

out[b,y,x] = bilinear_sample(imgs[b], y + dvfs[b,y,x,1], x + dvfs[b,y,x,0])
with zero-padding for out-of-bounds corners.

Strategy: pure data-parallel over batch (4 images per core).  On-chip the
per-pixel 2D gather is computed as a dense separable "hat" select-sum:

    out[y,x] = sum_r hat(dy-r) * sum_c hat(dx-c) * I[y+r, x+c]

where hat(t) = max(0, 1-|t|).  hat(dy-r)*hat(dx-c) is exactly the bilinear
corner weight for corner (y+r, x+c) and is nonzero only for the 4 corners
of each pixel, so summing over a tap set that covers all occurring
(floor(dy), floor(dx)) pairs is exact.  The tap set is computed on the host
from the actual displacement field (cheap histogram) so device work scales
with the true support of the data.

Implementation notes:
- Images are zero-padded on the host; each 128-row tile loads all n_r
  row-shifted copies of the image window with ONE overlapping 3D-AP DMA
  (engine APs require partition base in {0,32,64,96}, so row shifts cannot
  be partition-offset views; DMA has no such restriction).
- Hat coefficient planes are built on the Scalar (ACT) engine; tap
  multiply/adds run on the Vector (DVE) engine.
"""

import sys

sys.path.insert(0, "/opt/trn_rl_repo")

import dataclasses
from contextlib import ExitStack

import numpy as np

import concourse.bass as bass
import concourse.mybir as mybir
from concourse import tile
from concourse.bass_utils import run_bass_kernel_spmd

F32 = mybir.dt.float32
BF16 = mybir.dt.bfloat16
AF = mybir.ActivationFunctionType

N_CORES = 8

# compute taps in bf16 (2x DVE throughput); coefficients and accumulation
# structured so the result stays well inside a 2e-2 scale-relative gate
COMPUTE_BF16 = True


def _tap_sets(dvfs):
    """Exact tap support from the data: {r: sorted list of c}, for taps
    (r, c) such that some pixel has floor(dy) in {r-1, r} and
    floor(dx) in {c-1, c}."""
    fx = np.floor(dvfs[..., 0]).astype(np.int64).ravel()
    fy = np.floor(dvfs[..., 1]).astype(np.int64).ravel()
    lo = int(min(fx.min(), fy.min()))
    hi = int(max(fx.max(), fy.max()))
    n = hi - lo + 1
    joint = np.bincount((fy - lo) * n + (fx - lo), minlength=n * n) > 0
    joint = joint.reshape(n, n)
    # tap (r, c) needed iff joint[r - dr, c - dc] for (dr, dc) in {0,1}^2
    need = np.zeros((n + 1, n + 1), dtype=bool)
    for dr in (0, 1):
        for dc in (0, 1):
            need[dr : dr + n, dc : dc + n] |= joint
    taps = {}
    for ri in range(n + 1):
        cs = [int(ci) + lo for ci in np.nonzero(need[ri])[0]]
        if cs:
            taps[ri + lo] = cs
    return taps


def _split_multi_waits(nc):
    """This stack's walrus accepts at most one sync-wait per instruction;
    Tile emits several.  Hoist all-but-one wait onto preceding NoOps on the
    same engine queue (sequential execution makes that equivalent)."""
    for fn in nc.m.functions:
        for blk in fn.blocks:
            new_insts = []
            for ins in blk.instructions:
                si = ins.sync_info
                if si is not None and si.on_wait and len(si.on_wait) > 1:
                    waits = list(si.on_wait)
                    for w in waits[:-1]:
                        new_insts.append(
                            mybir.InstNoOp(
                                name=nc.get_next_instruction_name(),
                                engine=ins.engine,
                                bass_nofuse=True,
                                sync_info=mybir.SyncInfo(
                                    on_wait=[w], on_update=[]
                                ),
                            )
                        )
                    si.on_wait = [waits[-1]]
                new_insts.append(ins)
            blk.instructions = new_insts


def _build(taps, n_imgs, H, W, repeat=1):
    rs_used = sorted(taps.keys())
    rmin, rmax = min(rs_used), max(rs_used)
    # per-r contiguous c-ranges with even start: with pad_l and Wp even,
    # every bf16 tap view is then 4-byte aligned so the DVE 2x perf mode can
    # engage.  Extra taps added by widening are exactly zero-weighted.
    c_range = {}
    for r in rs_used:
        c0, c1 = min(taps[r]), max(taps[r])
        if c0 % 2 != 0:
            c0 -= 1
        c_range[r] = (c0, c1)
    cmin = min(c0 for c0, _ in c_range.values())
    cmax = max(c1 for _, c1 in c_range.values())
    pad_t, pad_b = max(0, -rmin), max(0, rmax)
    pad_l, pad_r = max(0, -cmin), max(0, cmax)
    n_r = rmax - rmin + 1
    Hp, Wp = H + pad_t + pad_b, W + pad_l + pad_r
    if Wp % 2 == 1:
        pad_r += 1
        Wp += 1
    cs_union = list(range(cmin, cmax + 1))
    c_idx = {c: i for i, c in enumerate(cs_union)}
    j_of_r = {r: r - rmin for r in rs_used}

    nc = bass.Bass()
    imgs = nc.dram_tensor("imgs", [n_imgs, Hp, Wp], F32, kind="ExternalInput")
    dvfs = nc.dram_tensor("dvfs", [n_imgs, H, 2 * W], F32, kind="ExternalInput")
    out = nc.dram_tensor("out", [n_imgs, H, W], F32, kind="ExternalOutput")

    # activation() biases must come from the const-AP registry
    consts = sorted({-float(v) for v in set(rs_used) | set(cs_union)} - {0.0, 1.0})
    for v in consts:
        t = nc.alloc_sbuf_tensor(f"const-f32-{v}", [128, 1], F32)
        nc.gpsimd.memset(t.ap(), v)
        nc.const_aps.aps[(F32, v)] = t.ap()
    nc.all_engine_barrier()

    with ExitStack() as ctx:
        tc = ctx.enter_context(tile.TileContext(nc))
        img_pool = ctx.enter_context(tc.tile_pool(name="img", bufs=1))
        dvf_pool = ctx.enter_context(tc.tile_pool(name="dvf", bufs=2))
        bp_pool = ctx.enter_context(tc.tile_pool(name="bp", bufs=1))
        av_pool = ctx.enter_context(tc.tile_pool(name="av", bufs=2))
        u_pool = ctx.enter_context(tc.tile_pool(name="u", bufs=2))
        h_pool = ctx.enter_context(tc.tile_pool(name="h", bufs=2))
        t_pool = ctx.enter_context(tc.tile_pool(name="t", bufs=1))
        o_pool = ctx.enter_context(tc.tile_pool(name="o", bufs=2))

        CD = BF16 if COMPUTE_BF16 else F32
        ib_pool = ctx.enter_context(tc.tile_pool(name="ib", bufs=1))

        for b in [bb for _ in range(repeat) for bb in range(n_imgs)]:
            for t0 in range(0, H, 128):
                # all n_r row-shifted image windows in one overlapping DMA:
                # IRALL[p, j*Wp + u] = imgs_padded[b, t0 + j + p, u]
                if not COMPUTE_BF16:
                    IRALL = img_pool.tile([128, n_r * Wp], F32, tag="IR")
                    src = dataclasses.replace(
                        imgs[b],
                        ap=[[Wp, 128], [Wp, n_r], [1, Wp]],
                        offset=b * Hp * Wp + t0 * Wp,
                    )
                    dst = IRALL[:, :].rearrange("p (j w) -> p j w", j=n_r)
                    nc.sync.dma_start(out=dst, in_=src)
                    IB = IRALL
                else:
                    # stage fp32 halves through a small buffer, convert to a
                    # bf16 copy (taps then run in the DVE 2x perf mode)
                    IB = ib_pool.tile([128, n_r * Wp], BF16, tag="IB")
                    nh = (n_r + 1) // 2
                    for hj, (j0, j1) in enumerate(((0, nh), (nh, n_r))):
                        if j1 <= j0:
                            continue
                        stage = img_pool.tile([128, nh * Wp], F32, tag="IR")
                        src = dataclasses.replace(
                            imgs[b],
                            ap=[[Wp, 128], [Wp, j1 - j0], [1, Wp]],
                            offset=b * Hp * Wp + (t0 + j0) * Wp,
                        )
                        dst = stage[:, 0 : (j1 - j0) * Wp].rearrange(
                            "p (j w) -> p j w", j=j1 - j0
                        )
                        nc.sync.dma_start(out=dst, in_=src)
                        nc.scalar.activation(
                            IB[:, j0 * Wp : j1 * Wp],
                            stage[:, 0 : (j1 - j0) * Wp],
                            AF.Copy,
                            bias=0.0,
                            scale=1.0,
                        )

                D = dvf_pool.tile([128, 2 * W], F32, tag="D")
                nc.sync.dma_start(out=D[:, :], in_=dvfs[b, t0 : t0 + 128, :])
                dx = D[:, 0 : 2 * W : 2]
                dy = D[:, 1 : 2 * W : 2]

                # horizontal hat planes: B_c = relu(1 - |dx - c|)
                BP = bp_pool.tile([128, W * len(cs_union)], CD, tag="BP")
                for c in cs_union:
                    i = c_idx[c]
                    U = u_pool.tile([128, W], F32, tag="U")
                    nc.scalar.activation(U[:, :], dx, AF.Abs, bias=-float(c), scale=1.0)
                    nc.scalar.activation(
                        BP[:, i * W : (i + 1) * W],
                        U[:, :],
                        AF.Relu,
                        bias=1.0,
                        scale=-1.0,
                    )

                OUT = o_pool.tile([128, W], F32, tag="OUT")
                max_nc = max(c1 - c0 + 1 for c0, c1 in c_range.values())
                first_r = True
                for r in rs_used:
                    # vertical hat plane: A_r = relu(1 - |dy - r|)
                    U = u_pool.tile([128, W], F32, tag="U")
                    AV = av_pool.tile([128, W], CD, tag="AV")
                    nc.scalar.activation(U[:, :], dy, AF.Abs, bias=-float(r), scale=1.0)
                    nc.scalar.activation(
                        AV[:, :], U[:, :], AF.Relu, bias=1.0, scale=-1.0
                    )

                    j = j_of_r[r]
                    c0, c1 = c_range[r]
                    k = c1 - c0 + 1
                    # all k taps of this row in ONE mult: in0 is an
                    # overlapping [128, k, W] view of the shifted image row,
                    # in1 the matching stack of hat planes
                    T = t_pool.tile([128, max_nc * W], CD, tag="T")
                    iv = dataclasses.replace(
                        IB[:, :],
                        ap=[[n_r * Wp, 128], [1, k], [1, W]],
                        offset=j * Wp + pad_l + c0,
                    )
                    bp = dataclasses.replace(
                        BP[:, :],
                        ap=[[len(cs_union) * W, 128], [W, k], [1, W]],
                        offset=c_idx[c0] * W,
                    )
                    tv = T[:, 0 : k * W].rearrange("p (k w) -> p k w", k=k)
                    nc.vector.tensor_tensor(tv, iv, bp, mybir.AluOpType.mult)
                    # fold the k product planes down to one
                    while k > 1:
                        if k % 2 == 1:
                            nc.vector.tensor_add(
                                T[:, 0:W], T[:, 0:W], T[:, (k - 1) * W : k * W]
                            )
                            k -= 1
                        else:
                            h = k // 2
                            nc.vector.tensor_add(
                                T[:, 0 : h * W],
                                T[:, 0 : h * W],
                                T[:, h * W : k * W],
                            )
                            k = h
                    # vertical blend on GpSimd (otherwise idle)
                    if first_r:
                        nc.gpsimd.tensor_mul(OUT[:, :], T[:, 0:W], AV[:, :])
                        first_r = False
                    else:
                        T2 = h_pool.tile([128, W], CD, tag="H")
                        nc.gpsimd.tensor_mul(T2[:, :], T[:, 0:W], AV[:, :])
                        nc.gpsimd.tensor_add(OUT[:, :], OUT[:, :], T2[:, :])

                nc.sync.dma_start(out=out[b, t0 : t0 + 128, :], in_=OUT[:, :])

    _split_multi_waits(nc)
    pads = (pad_t, pad_b, pad_l, pad_r)
    return nc, pads


def _make_runner(nc):
    """Mirror of bass2jax.run_bass_via_pjrt's multi-core path, but returning
    a reusable jitted callable so the NEFF can be re-executed for timing."""
    import jax
    from jax.experimental.shard_map import shard_map
    from jax.sharding import Mesh, PartitionSpec

    from concourse import bass2jax, mybir as mb

    bass2jax.install_neuronx_cc_hook()
    partition_name = nc.partition_id_tensor.name if nc.partition_id_tensor else None
    in_names, out_names, out_avals, zero_outs = [], [], [], []
    for alloc in nc.m.functions[0].allocations:
        if not isinstance(alloc, mb.MemoryLocationSet):
            continue
        name = alloc.memorylocations[0].name
        if alloc.kind == "ExternalInput":
            if name != partition_name:
                in_names.append(name)
        elif alloc.kind == "ExternalOutput":
            out_names.append(name)
            shape = tuple(alloc.tensor_shape)
            dtype = mb.dt.np(alloc.dtype)
            out_avals.append(jax.core.ShapedArray(shape, dtype))
            zero_outs.append(np.zeros(shape, dtype))
    n_params = len(in_names)
    n_outs = len(out_avals)
    all_in_names = list(in_names) + list(out_names)
    if partition_name is not None:
        all_in_names.append(partition_name)
    donate = tuple(range(n_params, n_params + n_outs))

    def _body(*args):
        operands = list(args)
        if partition_name is not None:
            operands.append(bass2jax.partition_id_tensor())
        outs = bass2jax._bass_exec_p.bind(
            *operands,
            out_avals=tuple(out_avals),
            in_names=tuple(all_in_names),
            out_names=tuple(out_names),
            lowering_input_output_aliases=(),
            sim_require_finite=True,
            sim_require_nnan=True,
            nc=nc,
        )
        return tuple(outs)

    devices = jax.devices()[:N_CORES]
    mesh = Mesh(np.asarray(devices), ("core",))
    in_specs = (PartitionSpec("core"),) * (n_params + n_outs)
    out_specs = (PartitionSpec("core"),) * n_outs
    # no donation: the kernel writes every output element, so the "zero"
    # output buffers can be staged on device once and reused across calls
    sharded = jax.jit(
        shard_map(
            _body, mesh=mesh, in_specs=in_specs, out_specs=out_specs, check_rep=False
        ),
        keep_unused=True,
    )

    from jax.sharding import NamedSharding

    shd = NamedSharding(mesh, PartitionSpec("core"))

    def run(in_maps, materialize=True, _staged={}):
        key = id(in_maps)
        if key not in _staged:
            per_core = [[np.asarray(m[name]) for name in in_names] for m in in_maps]
            concat_in = [
                np.concatenate([per_core[c][i] for c in range(N_CORES)], axis=0)
                for i in range(n_params)
            ]
            concat_zeros = [
                np.zeros((N_CORES * z.shape[0], *z.shape[1:]), z.dtype)
                for z in zero_outs
            ]
            _staged.clear()
            _staged[key] = [
                jax.device_put(a, shd) for a in concat_in + concat_zeros
            ]
            jax.block_until_ready(_staged[key])
        args = _staged[key]
        out_arrs = sharded(*args)
        jax.block_until_ready(out_arrs)
        if not materialize:
            return None
        return [
            {
                name: np.asarray(out_arrs[i]).reshape(N_CORES, *out_avals[i].shape)[c]
                for i, name in enumerate(out_names)
            }
            for c in range(N_CORES)
        ]

    return run


def _null_runner():
    """Tiny copy kernel used to measure fixed per-call dispatch overhead."""
    nc = bass.Bass()
    x = nc.dram_tensor("x", [128, 128], F32, kind="ExternalInput")
    y = nc.dram_tensor("y", [128, 128], F32, kind="ExternalOutput")
    from contextlib import ExitStack

    with ExitStack() as ctx:
        tc = ctx.enter_context(tile.TileContext(nc))
        pool = ctx.enter_context(tc.tile_pool(name="p", bufs=1))
        t = pool.tile([128, 128], F32)
        nc.sync.dma_start(out=t[:, :], in_=x[:, :])
        nc.sync.dma_start(out=y[:, :], in_=t[:, :])
    _split_multi_waits(nc)
    runner = _make_runner(nc)
    in_maps = [{"x": np.zeros((128, 128), np.float32)} for _ in range(N_CORES)]
    return runner, in_maps


def _prepare(imgs, dvfs, repeat=1):
    imgs = np.ascontiguousarray(np.asarray(imgs, dtype=np.float32))
    dvfs = np.ascontiguousarray(np.asarray(dvfs, dtype=np.float32))
    B, H, W = imgs.shape[0], imgs.shape[1], imgs.shape[2]
    n_per = B // N_CORES
    taps = _tap_sets(dvfs)
    nc, pads = _build(taps, n_per, H, W, repeat=repeat)
    pad_t, pad_b, pad_l, pad_r = pads
    imgs_p = np.zeros((B, H + pad_t + pad_b, W + pad_l + pad_r), np.float32)
    imgs_p[:, pad_t : pad_t + H, pad_l : pad_l + W] = imgs.reshape(B, H, W)
    dvfs3 = dvfs.reshape(B, H, 2 * W)
    in_maps = [
        {
            "imgs": imgs_p[i * n_per : (i + 1) * n_per],
            "dvfs": dvfs3[i * n_per : (i + 1) * n_per],
        }
        for i in range(N_CORES)
    ]
    return nc, in_maps, (B, H, W)


def _run(imgs, dvfs):
    nc, in_maps, (B, H, W) = _prepare(imgs, dvfs)
    runner = _make_runner(nc)
    results = runner(in_maps)
    outs = [np.asarray(m["out"]) for m in results]
    full = np.concatenate(outs, axis=0).reshape(B, H, W, 1)
    return full, runner, in_maps


def kernel(**inputs):
    full, _, _ = _run(inputs["imgs"], inputs["dvfs"])
    return full
